# revision 1
# baseline (speedup 1.0000x reference)
"""Trainium2 Bass kernel for nn_Conv2dGeneral (capsule-style 4x4-pose conv).

Math (from the reference):
  out[b,o,X,Y,u,w] = sum_{cin,kx,ky,v} Wm[(cin,kx,ky),o,u,v] * x[b,cin,2X+kx,2Y+ky,4v+w] + bias[o]

Mapped to the PE array as a single 1152-deep contraction:
  K = (cin, v)  x  9 accumulation chunks over (kx, ky)   [9 x 128]
  M = (o, u)                                              [128 PSUM partitions]
  N = (X, Y, w)                                           [676 per batch image]

Data-parallel across 8 NeuronCores on the batch dim (8 images per core).

Host-side prep: x is re-laid-out to [(b), (cin,v), (r,c,w)] so each core's
shard DMAs as fully-contiguous 12.5KB partition lines; the stride-2 im2col
window gather then happens for free inside the matmul moving-operand access
pattern (no patch materialization, each x byte read once from HBM).
"""

import numpy as np

B, CIN, COUT = 64, 32, 32
KK, STRIDE = 3, 2
WIN, HH = 28, 16
H = 4
WOUT = (WIN - KK) // STRIDE + 1  # 13
NCORES = 8
BPC = B // NCORES                # batches per core
RCW = WIN * WIN * H              # 3136 free elements per (cin,v) partition
NOUT = WOUT * WOUT * H           # 676 outputs per (o,u) partition per image
XSPLIT = ((0, 7), (7, 6))        # two PSUM groups: X rows [0,7) and [7,13)

_cache = {}


def _build_bass():
    """Raw-bass build (no Tile): this toolchain's walrus codegen allows only
    ONE sync-wait per instruction, so all cross-engine sync is explicit
    single-sem waits; ordering beyond that rides on hardware transitivity.

    Engines: SP triggers the 7 DMAs, PE runs 16 accumulation groups of 9
    matmuls (one per kernel tap), ACT evicts PSUM->SBUF adding the bias.
    """
    import concourse.bass as bass
    import concourse.mybir as mybir

    f32 = mybir.dt.float32
    f16 = mybir.dt.float16
    OB = 2                    # batches per output-DMA chunk
    NOC = BPC // OB           # 4 output chunks
    NG = 2 * BPC              # 16 PSUM accumulation groups
    GPO = 2 * OB              # groups per output chunk
    WARMUP = 24               # PE warm-up matmuls while x[0] streams in

    nc = bass.Bass()
    x_d = nc.declare_dram_parameter("x", [BPC, 128, RCW], f16, isOutput=False)
    w_d = nc.declare_dram_parameter("w", [128, 9 * 128], f16, isOutput=False)
    b_d = nc.declare_dram_parameter("b", [128, 1], f32, isOutput=False)
    o_d = nc.declare_dram_parameter("out", [NOC, 128, OB * NOUT], f32, isOutput=True)

    with (
        nc.sbuf_tensor([128, 9 * 128], f16) as wt,
        nc.sbuf_tensor([128, 1], f32) as bt,
        nc.sbuf_tensor([128, BPC, RCW], f16) as gt,
        nc.sbuf_tensor([128, NOC, OB * NOUT], f32) as ot,
        nc.psum_tensor([128, 8, 512], f32) as ps,
        nc.semaphore("wt_sem") as wt_sem,
        nc.semaphore("bias_sem") as bias_sem,
        nc.semaphore("g_sem0") as g_sem0,
        nc.semaphore("g_sem1") as g_sem1,
        nc.semaphore("g_sem2") as g_sem2,
        nc.semaphore("g_sem3") as g_sem3,
        nc.semaphore("g_sem4") as g_sem4,
        nc.semaphore("g_sem5") as g_sem5,
        nc.semaphore("g_sem6") as g_sem6,
        nc.semaphore("g_sem7") as g_sem7,
        nc.semaphore("pe_sem") as pe_sem,
        nc.semaphore("act_sem") as act_sem,
        nc.semaphore("out_sem") as out_sem,
        nc.Block() as block,
    ):
        g_sems = [g_sem0, g_sem1, g_sem2, g_sem3, g_sem4, g_sem5, g_sem6, g_sem7]
        wtr = wt[:, :].rearrange("p (k m) -> p k m", k=9)

        @block.sync
        def _(sync):
            sync.dma_start(wt[:, :], w_d[:, :]).then_inc(wt_sem, 16)
            sync.dma_start(bt[:, :], b_d[:, :]).then_inc(bias_sem, 16)
            for b in range(BPC):
                sync.dma_start(gt[:, b, :], x_d[b]).then_inc(g_sems[b], 16)
            sync.wait_ge(out_sem, 16 * NOC)

        @block.tensor
        def _(tensor):
            tensor.wait_ge(wt_sem, 16)
            # Warm the PE HAM clock gate (cold = 1.2 GHz) while x streams in.
            for i in range(WARMUP):
                tensor.matmul(
                    ps[:, 7, :128], wt[:, :128], wt[:, :128], start=True, stop=True
                )
            for j in range(NG):
                b, half = divmod(j, 2)
                if half == 0:
                    tensor.wait_ge(g_sems[b], 16)
                if j >= 8:
                    # PSUM bank j%8 is free once ACT drained group j-8
                    tensor.wait_ge(act_sem, j - 7)
                X0, nX = XSPLIT[half]
                gr = gt[:, b, :].rearrange("p (r c w) -> p r c w", r=WIN, c=WIN)
                for kk in range(9):
                    kx, ky = divmod(kk, 3)
                    rhs = gr[
                        :,
                        2 * X0 + kx : 2 * X0 + kx + 2 * nX - 1 : 2,
                        ky : ky + 2 * WOUT - 1 : 2,
                        :,
                    ]
                    mm = tensor.matmul(
                        ps[:, j % 8, : nX * WOUT * H],
                        wtr[:, kk, :],
                        rhs,
                        start=(kk == 0),
                        stop=(kk == 8),
                    )
                mm.then_inc(pe_sem, 1)

        @block.scalar
        def _(scalar):
            scalar.wait_ge(bias_sem, 16)
            for j in range(NG):
                b, half = divmod(j, 2)
                X0, nX = XSPLIT[half]
                oc, obi = divmod(b, OB)
                off = obi * NOUT + X0 * WOUT * H
                scalar.wait_ge(pe_sem, j + 1)
                scalar.activation(
                    ot[:, oc, off : off + nX * WOUT * H],
                    ps[:, j % 8, : nX * WOUT * H],
                    mybir.ActivationFunctionType.Identity,
                    bias=bt[:, :],
                ).then_inc(act_sem, 1)
                if j % GPO == GPO - 1:
                    # output chunk complete; ship it from the ACT ring
                    scalar.dma_start(o_d[j // GPO], ot[:, j // GPO, :]).then_inc(
                        out_sem, 16
                    )

    return nc


def _prep_inputs(x, W, bias):
    # x: (B, CIN, 28, 28, 16) -> xp[b, cin*4+v, (r*28+c)*4+w] = x[b,cin,r,c,4v+w]
    # fp16: PE runs fp32 matmuls as LOW_HIGH double passes; fp16 is single-pass
    # with fast-weight-load, and halves the dominant HBM traffic. Max rel err
    # ~3e-4 at this contraction depth (fp32 PSUM accumulation).
    xp = np.ascontiguousarray(
        x.reshape(B, CIN, WIN, WIN, H, H).transpose(0, 1, 4, 2, 3, 5)
    ).reshape(B, CIN * H, RCW).astype(np.float16)
    # W: (1, 288, 32, 1, 1, 4, 4); p = cin*9 + kx*3 + ky
    # wt_sb[cin*4+v, kk*128 + o*4+u] = Wm[cin*9+kk, o, u, v]
    Wm = np.asarray(W, dtype=np.float32).reshape(CIN, KK * KK, COUT, H, H)
    wt_sb = np.ascontiguousarray(
        Wm.transpose(0, 4, 1, 2, 3)  # cin, v, kk, o, u
    ).reshape(128, 9 * 128).astype(np.float16)
    bias_v = np.ascontiguousarray(
        np.repeat(np.asarray(bias, dtype=np.float32).reshape(COUT), H)
    ).reshape(128, 1)
    return xp, wt_sb, bias_v


def _shard_x(xp, core):
    # per-core input: [BPC, 128, RCW] fp16
    return np.ascontiguousarray(xp[core * BPC : (core + 1) * BPC])


def _unchunk_out(dev_out, ob=2):
    # dev_out: (BPC//ob, 128, ob*NOUT) -> (BPC, 128, NOUT)
    return (
        dev_out.reshape(BPC // ob, 128, ob, NOUT)
        .transpose(0, 2, 1, 3)
        .reshape(BPC, 128, NOUT)
    )


def _unprep_output(full):
    # full: (B, 128, NOUT) with partition o*4+u, free (X, Y, w)
    out = (
        full.reshape(B, COUT, H, WOUT, WOUT, H)
        .transpose(0, 1, 3, 4, 2, 5)
        .reshape(B, COUT, WOUT, WOUT, HH)
    )
    return np.ascontiguousarray(out)


def run_device(in_maps, trace=False, tmpdir=None):
    from concourse.bass_utils import run_bass_kernel_spmd

    if "nc" not in _cache:
        _cache["nc"] = _build_bass()
    return run_bass_kernel_spmd(
        _cache["nc"], in_maps, list(range(NCORES)), trace=trace, tmpdir=tmpdir
    )


def kernel(x, W, bias):
    x = np.asarray(x, dtype=np.float32)
    xp, wt_sb, bias_v = _prep_inputs(x, W, bias)
    in_maps = [
        {"x": _shard_x(xp, i), "w": wt_sb, "b": bias_v} for i in range(NCORES)
    ]
    res = run_device(in_maps, trace=False)
    full = np.concatenate(
        [_unchunk_out(res.results[i]["out"]) for i in range(NCORES)], axis=0
    )
    return _unprep_output(full)



# revision 15
# speedup vs baseline: 1.0824x; 1.0824x over previous
"""Trainium2 Bass kernel for nn_Conv2dGeneral (capsule-style 4x4-pose conv).

Math (from the reference):
  out[b,o,X,Y,u,w] = sum_{cin,kx,ky,v} Wm[(cin,kx,ky),o,u,v] * x[b,cin,2X+kx,2Y+ky,4v+w] + bias[o]

Mapped to the PE array as a single 1152-deep contraction:
  K = (cin, v)  x  9 accumulation chunks over (kx, ky)   [9 x 128]
  M = (o, u)                                              [128 PSUM partitions]
  N = (X, Y, w)                                           [676 per batch image]

Data-parallel across 8 NeuronCores on the batch dim (8 images per core).

Pipelining layout: bias + weights + all 8 images are packed into ONE fp16
DRAM buffer per core, streamed by 17 column-range DMA chunks so compute on
image 0 starts as soon as its first rows land. Row/col 27 of x are dead
(stride-2 3-tap windows over 28 touch only 0..26) and are dropped host-side.
The PE warms its HAM clock gate on garbage matmuls while the first chunk is
in flight; outputs are evicted per-image in fp16 and widened on the host.
"""

import numpy as np

B, CIN, COUT = 64, 32, 32
KK, STRIDE = 3, 2
WIN, HH = 28, 16
H = 4
WU = 27                          # used rows/cols (row 27 never read)
WOUT = (WIN - KK) // STRIDE + 1  # 13
NCORES = 8
BPC = B // NCORES                # batches per core
RCW = WU * WU * H                # 2916 free elements per (cin,v) partition
NOUT = WOUT * WOUT * H           # 676 outputs per (o,u) partition per image
XSPLIT = ((0, 4), (4, 4), (8, 5))  # 3 PSUM groups per image on X rows
GPB = len(XSPLIT)                # groups per image
NG = GPB * BPC                   # 24 accumulation groups
WARMUP = 30                      # PE warm-up matmuls while chunk 0 streams in

OFF_W = 0                        # [wt(1152) | img0..7(2916 each)]
OFF_X = OFF_W + 9 * 128
NELEM = OFF_X + BPC * RCW

# (elem_start, elem_end) DMA chunks: img0 in row-thirds (matching XSPLIT
# needs: rows [0,9), [9,17), [17,27)), imgs 1..7 whole. Each chunk gets its
# OWN completion semaphore: a single cumulative sem is racy because the 16
# per-SDMA-engine increments of back-to-back DMAs interleave, so sem>=16
# does not imply the FIRST dma finished. Per-engine FIFO order does make
# "chunk c done" imply all earlier chunks done, so one wait per group works.
_CHUNKS = [
    (0, OFF_X + 9 * WU * H),
    (OFF_X + 9 * WU * H, OFF_X + 17 * WU * H),
    (OFF_X + 17 * WU * H, OFF_X + RCW),
]
for _b in range(1, BPC):
    _o = OFF_X + _b * RCW
    _CHUNKS.append((_o, _o + RCW))
NCHUNK = len(_CHUNKS)


def _chunk_needed(b, t):
    # index of the last DMA chunk group (b, t) requires
    if b == 0:
        return t
    return b + 2


_cache = {}


def _build_bass():
    """Raw-bass build (no Tile): this toolchain's walrus codegen allows only
    ONE sync-wait per instruction, so all cross-engine sync is explicit
    single-sem waits; ordering beyond that rides on hardware transitivity.

    Engines: SP triggers the 17 input DMAs, PE runs 24 accumulation groups
    of 9 matmuls (one per kernel tap), ACT evicts PSUM->SBUF in fp16 adding
    the bias and ships per-image output DMAs.
    """
    import concourse.bass as bass
    import concourse.mybir as mybir

    f32 = mybir.dt.float32
    f16 = mybir.dt.float16

    from contextlib import ExitStack

    nc = bass.Bass()
    xin = nc.declare_dram_parameter("xin", [128, NELEM], f16, isOutput=False)
    o_d = nc.declare_dram_parameter("out", [BPC, 128, NOUT], f16, isOutput=True)

    with (
        ExitStack() as stack,
        nc.sbuf_tensor([128, NELEM], f16) as allt,
        nc.sbuf_tensor([128, BPC, NOUT], f16) as ot,
        nc.psum_tensor([128, 8, 512], f32) as ps,
        nc.semaphore("pe_sem") as pe_sem,
        nc.semaphore("act_sem") as act_sem,
        nc.semaphore("out_sem") as out_sem,
        nc.semaphore("warm_sem") as warm_sem,
        nc.Block() as block,
    ):
        c_sems = [
            stack.enter_context(nc.semaphore(f"c_sem{i}")) for i in range(NCHUNK)
        ]
        wtr = allt[:, OFF_W : OFF_W + 9 * 128].rearrange("p (k m) -> p k m", k=9)

        @block.sync
        def _(sync):
            for c, (a0, a1) in enumerate(_CHUNKS):
                sync.dma_start(allt[:, a0:a1], xin[:, a0:a1]).then_inc(c_sems[c], 16)
            sync.wait_ge(out_sem, 16 * BPC)

        @block.vector
        def _(vector):
            # Zero the warm-up operand region: reading never-written SBUF
            # faults the exec unit on hardware (and trips the sim).
            vector.memset(ot[:, 0, :128], 0).then_inc(warm_sem, 1)

        @block.tensor
        def _(tensor):
            # Warm the PE HAM clock gate (cold = 1.2 GHz) on zeros while
            # chunk 0 (weights+img0 rows 0-8) streams in.
            tensor.wait_ge(warm_sem, 1)
            for i in range(WARMUP):
                tensor.matmul(
                    ps[:, 7, :128], ot[:, 0, :128], ot[:, 0, :128],
                    start=True, stop=True,
                )
            prev_need = -1
            for j in range(NG):
                b, t = divmod(j, GPB)
                need = _chunk_needed(b, t)
                if need > prev_need:
                    tensor.wait_ge(c_sems[need], 16)
                    prev_need = need
                if j >= 8:
                    # PSUM bank j%8 is free once ACT drained group j-8
                    tensor.wait_ge(act_sem, j - 7)
                X0, nX = XSPLIT[t]
                gr = allt[:, OFF_X + b * RCW : OFF_X + (b + 1) * RCW].rearrange(
                    "p (r c w) -> p r c w", r=WU, c=WU
                )
                for kk in range(9):
                    kx, ky = divmod(kk, 3)
                    rhs = gr[
                        :,
                        2 * X0 + kx : 2 * X0 + kx + 2 * nX - 1 : 2,
                        ky : ky + 2 * WOUT - 1 : 2,
                        :,
                    ]
                    mm = tensor.matmul(
                        ps[:, j % 8, : nX * WOUT * H],
                        wtr[:, kk, :],
                        rhs,
                        start=(kk == 0),
                        stop=(kk == 8),
                    )
                mm.then_inc(pe_sem, 1)

        @block.scalar
        def _(scalar):
            for j in range(NG):
                b, t = divmod(j, GPB)
                X0, nX = XSPLIT[t]
                off = X0 * WOUT * H
                scalar.wait_ge(pe_sem, j + 1)
                scalar.activation(
                    ot[:, b, off : off + nX * WOUT * H],
                    ps[:, j % 8, : nX * WOUT * H],
                    mybir.ActivationFunctionType.Copy,
                ).then_inc(act_sem, 1)
                if t == GPB - 1:
                    # image complete; ship it from the ACT ring (the wait
                    # orders the async DMA read after this engine's writes)
                    scalar.wait_ge(act_sem, j + 1)
                    scalar.dma_start(o_d[b], ot[:, b, :]).then_inc(out_sem, 16)

    return nc


def _prep_inputs(x, W, bias):
    """Build per-core [128, NELEM] fp16 input buffers.

    fp16: PE runs fp32 matmuls as LOW_HIGH double passes; fp16 is single-pass
    with fast-weight-load, and halves the dominant HBM traffic. Max rel err
    ~3e-4 at this contraction depth (fp32 PSUM accumulation).
    """
    x = np.asarray(x, dtype=np.float32)
    # xp[b, cin*4+v, (r*27+c)*4+w] = x[b,cin,r,c,4v+w], r/c < 27
    xp = np.ascontiguousarray(
        x.reshape(B, CIN, WIN, WIN, H, H)[:, :, :WU, :WU]
        .transpose(0, 1, 4, 2, 3, 5)
    ).reshape(B, CIN * H, RCW).astype(np.float16)
    # W: (1, 288, 32, 1, 1, 4, 4); p = cin*9 + kx*3 + ky
    # wt_sb[cin*4+v, kk*128 + o*4+u] = Wm[cin*9+kk, o, u, v]
    Wm = np.asarray(W, dtype=np.float32).reshape(CIN, KK * KK, COUT, H, H)
    wt_sb = np.ascontiguousarray(
        Wm.transpose(0, 4, 1, 2, 3)  # cin, v, kk, o, u
    ).reshape(128, 9 * 128).astype(np.float16)
    bufs = []
    for core in range(NCORES):
        shard = xp[core * BPC : (core + 1) * BPC]  # (BPC, 128, RCW)
        bufs.append(
            np.ascontiguousarray(
                np.concatenate(
                    [wt_sb, shard.transpose(1, 0, 2).reshape(128, BPC * RCW)],
                    axis=1,
                )
            )
        )
    return bufs


def _make_in_maps(x, W, bias):
    return [{"xin": buf} for buf in _prep_inputs(x, W, bias)]


def _unprep_output(full, bias):
    # full: (B, 128, NOUT) fp16 with partition o*4+u, free (X, Y, w).
    # Bias (a per-channel constant) is added host-side to keep the device
    # eviction a plain fp16 Copy.
    out = (
        full.astype(np.float32)
        .reshape(B, COUT, H, WOUT, WOUT, H)
        .transpose(0, 1, 3, 4, 2, 5)
        .reshape(B, COUT, WOUT, WOUT, HH)
    )
    out += np.asarray(bias, dtype=np.float32).reshape(1, COUT, 1, 1, 1)
    return np.ascontiguousarray(out)


def run_device(in_maps, trace=False, tmpdir=None):
    from concourse.bass_utils import run_bass_kernel_spmd

    if "nc" not in _cache:
        _cache["nc"] = _build_bass()
    return run_bass_kernel_spmd(
        _cache["nc"], in_maps, list(range(NCORES)), trace=trace, tmpdir=tmpdir
    )


def kernel(x, W, bias):
    in_maps = _make_in_maps(x, W, bias)
    res = run_device(in_maps, trace=False)
    full = np.concatenate(
        [np.asarray(res.results[i]["out"]) for i in range(NCORES)], axis=0
    )
    return _unprep_output(full, bias)
